# revision 1
# baseline (speedup 1.0000x reference)
"""Multi-head attention (B=2, T=4096, C=768, H=12, Dk=64) on 8 trn2 NeuronCores.

Sharding: core c -> batch b = c//4, head-group g = c%4 (3 heads each).
Megatron-style: each core computes qkv projection for its 3 heads, full
attention for those heads, and a row-parallel partial of the output
projection. Host sums the 4 partials per batch (+ bias, folded into the
g==0 core's partial on device).

Device algorithm (per core), everything fp32:
  - qkT[c, t] feature-major via matmul(lhsT=Wqk_cols, rhs=xT) with
    column packing [q0 q1 | k0 k1 | q2 k2] so head0 lives on SBUF
    partitions 0-63 and head1 on 64-127 (natural PE row-tiling pairs),
    plus a swapped duplicate slot o3 = [k2 | q2] so head2 pairs across
    alternating tk-blocks.
  - V token-major [t, 64] per head with an appended ones column ->
    attention matmul also accumulates the softmax denominator.
  - attention in S^T layout: ST[tk,tq-blk] = KT^T@QT, exp on ACT engine
    (scale=1/8 fused into the activation), OT^T[dv,tq] += V_aug^T@expST.
  - normalize with DVE reciprocal + gpsimd partition broadcast.
  - out projection from OT^T (feature-major) with Wout rows.
"""

import os
import sys
from contextlib import ExitStack

import numpy as np

for _p in ("/opt/trn_rl_repo", "/root/.axon_site/_ro/trn_rl_repo"):
    if os.path.isdir(_p) and _p not in sys.path:
        sys.path.append(_p)

import concourse.bass as bass
import concourse.mybir as mybir
import concourse.tile as tile
from concourse import bacc
from concourse.bass import ts
from concourse.bass_utils import run_bass_kernel_spmd

F32 = mybir.dt.float32
F32R = mybir.dt.float32r
F16 = mybir.dt.float16

B, T, C = 2, 4096, 768
H, DK = 12, 64
N_CORES = 8
HPC = 3  # heads per core
GQ = 512  # q-block (matmul free dim)
NTQ = T // GQ  # 8 q-blocks
NTK = T // 128  # 32 tk-blocks


def _build_program(debug_taps=False):
    nc = bacc.Bacc("TRN2", target_bir_lowering=False, debug=False)

    xT = nc.dram_tensor("xT", [C, T], F32R, kind="ExternalInput").ap()
    wqk = nc.dram_tensor("wqk", [C, 384], F32R, kind="ExternalInput").ap()
    bqk = nc.dram_tensor("bqk", [384], F32, kind="ExternalInput").ap()
    wv = nc.dram_tensor("wv", [C, 192], F32R, kind="ExternalInput").ap()
    bv = nc.dram_tensor("bv", [192], F32, kind="ExternalInput").ap()
    wout = nc.dram_tensor("wout", [192, C], F16, kind="ExternalInput").ap()
    bout = nc.dram_tensor("bout", [C], F32, kind="ExternalInput").ap()
    ones = nc.dram_tensor("ones", [NTK * HPC], F16, kind="ExternalInput").ap()
    y = nc.dram_tensor("y", [T, C], F32, kind="ExternalOutput").ap()

    xT3 = xT.rearrange("(o p) t -> p o t", p=128)  # [128, 6, 4096]
    y3 = y.rearrange("(n p) e -> p n e", p=128)  # [128, 32, 768]

    dbg = {}
    if debug_taps:
        for name, shape in [
            ("dbg_qkT", [128, 4, 512]),
            ("dbg_vaug", [128, 3, 65]),
            ("dbg_est", [128, 2, 512]),
            ("dbg_ot", [65, 512]),
            ("dbg_rc", [1, 512]),
            ("dbg_rb", [64, 512]),
            ("dbg_otn", [64, 512]),
        ]:
            dbg[name] = nc.dram_tensor(name, shape, F32, kind="ExternalOutput").ap()

    with tile.TileContext(nc) as tc, ExitStack() as ctx:
        sb = ctx.enter_context(tc.tile_pool(name="persist", bufs=1))

        # --- weights / biases ---
        wqk_sb = sb.tile([128, 6, 384], F32R)
        nc.sync.dma_start(wqk_sb[:], wqk.rearrange("(o p) c -> p o c", p=128))
        wv_sb = sb.tile([128, 6, 192], F32R)
        nc.sync.dma_start(wv_sb[:], wv.rearrange("(o p) c -> p o c", p=128))
        wout_sb = sb.tile([64, 3, C], F16)
        nc.sync.dma_start(wout_sb[:], wout.rearrange("(h p) e -> p h e", p=64))
        bqk_sb = sb.tile([128, 3], F32)
        nc.sync.dma_start(bqk_sb[:], bqk.rearrange("(o p) -> p o", p=128))
        bv_bc = sb.tile([128, 192], F32)
        nc.sync.dma_start(bv_bc[:], bv[None, :].to_broadcast((128, 192)))
        bout_bc = sb.tile([128, C], F32)
        nc.sync.dma_start(bout_bc[:], bout[None, :].to_broadcast((128, C)))

        # --- persistent activations ---
        # qkT slots: o0=[q0|q1] o1=[k0|k1] o2=[q2|k2] o3=[k2|q2]
        qkT = sb.tile([128, 4, T], F16)
        # V per tk-block per head, token-major, with ones col at [.., 64]
        vaug = sb.tile([128, NTK, HPC, 65], F16)
        nc.sync.dma_start(
            vaug[:, :, :, 64:65], ones[None, :].to_broadcast((128, NTK * HPC))
        )

        # --- prologue: qkv projections ---
        with (
            tc.tile_pool(name="psA", bufs=2, space="PSUM") as psA,
            tc.tile_pool(name="xin", bufs=2) as xin,
        ):
            for tb in range(NTQ):  # qk proj over 512-col blocks of t
                xt = xin.tile([128, 6, GQ], F32R, tag="xqk")
                nc.sync.dma_start(xt[:], xT3[:, :, ts(tb, GQ)])
                for cb in range(3):
                    ps = psA.tile([128, GQ], F32, tag="qk")
                    for d in range(6):
                        nc.tensor.matmul(
                            ps[:],
                            (wqk_sb[:, d, ts(cb, 128)]),
                            (xt[:, d, :]),
                            start=(d == 0),
                            stop=(d == 5),
                        )
                    nc.vector.tensor_scalar_add(
                        qkT[:, cb, ts(tb, GQ)], ps[:], bqk_sb[:, cb : cb + 1]
                    )
                # o3 = swap halves of o2 (k2|q2)
                nc.sync.dma_start(qkT[0:64, 3, ts(tb, GQ)], qkT[64:128, 2, ts(tb, GQ)])
                nc.sync.dma_start(qkT[64:128, 3, ts(tb, GQ)], qkT[0:64, 2, ts(tb, GQ)])

            for tb in range(NTK):  # v proj over 128-row blocks of t
                xv = xin.tile([128, 6, 128], F32R, tag="xv")
                nc.sync.dma_start(xv[:], xT3[:, :, ts(tb, 128)])
                psv = psA.tile([128, 192], F32, tag="v")
                for d in range(6):
                    nc.tensor.matmul(
                        psv[:],
                        (xv[:, d, :]),
                        (wv_sb[:, d, :]),
                        start=(d == 0),
                        stop=(d == 5),
                    )
                nc.vector.tensor_tensor(
                    vaug[:, tb, :, 0:64],
                    psv[:].rearrange("p (h d) -> p h d", d=64),
                    bv_bc[:].rearrange("p (h d) -> p h d", d=64),
                    mybir.AluOpType.add,
                )

        # --- attention + output projection ---
        with (
            tc.tile_pool(name="psST", bufs=2, space="PSUM") as psST,
            tc.tile_pool(name="psOT", bufs=3, space="PSUM") as psOT,
            tc.tile_pool(name="psY", bufs=1, space="PSUM") as psY,
            tc.tile_pool(name="estp", bufs=3) as estp,
            tc.tile_pool(name="otp", bufs=2) as otp,
            tc.tile_pool(name="smallp", bufs=3) as smallp,
            tc.tile_pool(name="yp", bufs=2) as yp,
            tc.tile_pool(name="dramp", bufs=3, space="DRAM") as dramp,
        ):

            def normalize(ps_ot, ot_dst, tap=False):
                rc = smallp.tile([1, GQ], F32, tag="rc")
                nc.vector.reciprocal(rc[:], ps_ot[64:65, :])
                dn = dramp.tile([GQ], F32, tag="dn")
                nc.sync.dma_start(dn[:], rc[:])
                rb = smallp.tile([64, GQ], F32, tag="rb")
                nc.sync.dma_start(rb[:], dn[None, :].to_broadcast((64, GQ)))
                nc.vector.tensor_tensor(
                    ot_dst, ps_ot[0:64, :], rb[:], mybir.AluOpType.mult
                )
                if tap:
                    ots = smallp.tile([65, GQ], F32, tag="dbg_ots")
                    nc.vector.tensor_copy(ots[:], ps_ot[:])
                    nc.sync.dma_start(dbg["dbg_ot"], ots[:])
                    nc.sync.dma_start(dbg["dbg_rc"], rc[:])
                    nc.sync.dma_start(dbg["dbg_rb"], rb[:])
                    nc.sync.dma_start(dbg["dbg_otn"], ot_dst)

            for tq in range(NTQ):
                ot_tile = otp.tile([64, HPC, GQ], F16, tag="ot_sb")

                # -- heads 0,1 (paired on PE rows lo/hi) --
                ps_ot0 = psOT.tile([65, GQ], F32, tag="ot")
                ps_ot1 = psOT.tile([65, GQ], F32, tag="ot")
                for g in range(NTK):
                    st = psST.tile([128, 2, GQ], F32, tag="st")
                    nc.tensor.matmul(
                        st[:, 0, :],
                        (qkT[0:64, 1, ts(g, 128)]),
                        (qkT[0:64, 0, ts(tq, GQ)]),
                        start=True,
                        stop=True,
                    )
                    nc.tensor.matmul(
                        st[:, 1, :],
                        (qkT[64:128, 1, ts(g, 128)]),
                        (qkT[64:128, 0, ts(tq, GQ)]),
                        start=True,
                        stop=True,
                    )
                    est = estp.tile([128, 2, GQ], F16, tag="est")
                    nc.scalar.activation(
                        est[:], st[:], mybir.ActivationFunctionType.Exp, scale=0.125
                    )
                    if debug_taps and tq == 0 and g == 0:
                        nc.sync.dma_start(dbg["dbg_est"], est[:])
                        nc.sync.dma_start(dbg["dbg_qkT"], qkT[:, :, 0:512])
                        nc.sync.dma_start(dbg["dbg_vaug"], vaug[:, 0, :, :])
                    nc.tensor.matmul(
                        ps_ot0[:],
                        (vaug[:, g, 0, :]),
                        (est[:, 0, :]),
                        start=(g == 0),
                        stop=(g == NTK - 1),
                    )
                    nc.tensor.matmul(
                        ps_ot1[:],
                        (vaug[:, g, 1, :]),
                        (est[:, 1, :]),
                        start=(g == 0),
                        stop=(g == NTK - 1),
                    )
                normalize(ps_ot0, ot_tile[:, 0, :], tap=(debug_taps and tq == 0))
                normalize(ps_ot1, ot_tile[:, 1, :])

                # -- head 2 (paired across even/odd tk-blocks) --
                ps_ot2 = psOT.tile([65, GQ], F32, tag="ot")
                for g2 in range(NTK // 2):
                    ge, go = 2 * g2, 2 * g2 + 1
                    st = psST.tile([128, 2, GQ], F32, tag="st")
                    nc.tensor.matmul(
                        st[:, 0, :],
                        (qkT[0:64, 3, ts(ge, 128)]),
                        (qkT[0:64, 2, ts(tq, GQ)]),
                        start=True,
                        stop=True,
                    )
                    nc.tensor.matmul(
                        st[:, 1, :],
                        (qkT[64:128, 2, ts(go, 128)]),
                        (qkT[64:128, 3, ts(tq, GQ)]),
                        start=True,
                        stop=True,
                    )
                    est = estp.tile([128, 2, GQ], F16, tag="est")
                    nc.scalar.activation(
                        est[:], st[:], mybir.ActivationFunctionType.Exp, scale=0.125
                    )
                    nc.tensor.matmul(
                        ps_ot2[:],
                        (vaug[:, ge, 2, :]),
                        (est[:, 0, :]),
                        start=(g2 == 0),
                        stop=False,
                    )
                    nc.tensor.matmul(
                        ps_ot2[:],
                        (vaug[:, go, 2, :]),
                        (est[:, 1, :]),
                        start=False,
                        stop=(g2 == NTK // 2 - 1),
                    )
                normalize(ps_ot2, ot_tile[:, 2, :])

                # -- output projection for this q-block --
                for tsub in range(GQ // 128):
                    y_sb = yp.tile([128, C], F32, tag="y_sb")
                    for nh in range(2):
                        py = psY.tile([128, 384], F32, tag="y")
                        for h in range(HPC):
                            nc.tensor.matmul(
                                py[:],
                                (ot_tile[:, h, ts(tsub, 128)]),
                                (wout_sb[:, h, ts(nh, 384)]),
                                start=(h == 0),
                                stop=(h == HPC - 1),
                            )
                        nc.vector.tensor_tensor(
                            y_sb[:, ts(nh, 384)],
                            py[:],
                            bout_bc[:, ts(nh, 384)],
                            mybir.AluOpType.add,
                        )
                    nc.sync.dma_start(y3[:, tq * (GQ // 128) + tsub, :], y_sb[:])

    nc.compile()
    return nc


_PROGRAM = None


def _get_program():
    global _PROGRAM
    if _PROGRAM is None:
        _PROGRAM = _build_program()
    return _PROGRAM


def _make_in_maps(x, W_qkv, b_qkv, W_out, b_out):
    x = np.asarray(x, dtype=np.float32)
    W_qkv = np.asarray(W_qkv, dtype=np.float32)
    b_qkv = np.asarray(b_qkv, dtype=np.float32)
    W_out = np.asarray(W_out, dtype=np.float32)
    b_out = np.asarray(b_out, dtype=np.float32)

    xT_b = [np.ascontiguousarray(x[b].T) for b in range(B)]
    in_maps = []
    for c in range(N_CORES):
        b, g = divmod(c, 4)
        h0 = HPC * g

        def qcol(h):
            return slice(h * DK, (h + 1) * DK)

        def kcol(h):
            return slice(C + h * DK, C + (h + 1) * DK)

        wqk_c = np.concatenate(
            [
                W_qkv[:, qcol(h0)],
                W_qkv[:, qcol(h0 + 1)],
                W_qkv[:, kcol(h0)],
                W_qkv[:, kcol(h0 + 1)],
                W_qkv[:, qcol(h0 + 2)],
                W_qkv[:, kcol(h0 + 2)],
            ],
            axis=1,
        )
        bqk_c = np.concatenate(
            [
                b_qkv[qcol(h0)],
                b_qkv[qcol(h0 + 1)],
                b_qkv[kcol(h0)],
                b_qkv[kcol(h0 + 1)],
                b_qkv[qcol(h0 + 2)],
                b_qkv[kcol(h0 + 2)],
            ]
        )
        vs = slice(2 * C + h0 * DK, 2 * C + (h0 + HPC) * DK)
        in_maps.append(
            {
                "xT": np.ascontiguousarray(xT_b[b]),
                "wqk": np.ascontiguousarray(wqk_c),
                "bqk": np.ascontiguousarray(bqk_c),
                "wv": np.ascontiguousarray(W_qkv[:, vs]),
                "bv": np.ascontiguousarray(b_qkv[vs]),
                "wout": np.ascontiguousarray(W_out[h0 * DK : (h0 + HPC) * DK, :]).astype(np.float16),
                "bout": (b_out if g == 0 else np.zeros_like(b_out)).copy(),
                "ones": np.ones(NTK * HPC, dtype=np.float16),
            }
        )
    return in_maps


def _assemble(results):
    out = np.zeros((B, T, C), dtype=np.float32)
    for c in range(N_CORES):
        out[c // 4] += results[c]["y"]
    return out


def kernel_run(inputs, trace=False):
    """Returns (full_output [B,T,C] fp32, exec_time_ns or None)."""
    nc = _get_program()
    in_maps = _make_in_maps(**inputs)
    res = run_bass_kernel_spmd(
        nc, in_maps, core_ids=list(range(N_CORES)), trace=trace
    )
    return _assemble(res.results), res.exec_time_ns


def kernel(**inputs):
    out, _ = kernel_run(inputs)
    return out



# revision 12
# speedup vs baseline: 1.3692x; 1.3692x over previous
"""Multi-head attention (B=2, T=4096, C=768, H=12, Dk=64) on 8 trn2 NeuronCores.

Sharding: core c -> batch b = c//4, head-group g = c%4 (3 heads each).
Each core: qkv projection for its 3 heads, full attention, row-parallel
partial of the output projection. Host sums the 4 partials per batch.

v2 pipeline (per core), measurement-driven design:
  - qkv proj in bf16 (PE streams 1 col/cycle regardless of dtype; bf16
    halves x DMA).  qk feature-major fp16 [128|64, T]; v token-major
    fp8e4 with an appended ones column (softmax denominator rides the
    PV matmul).
  - scores S^T = K^T Q in fp16, [128 keys, 2, 512 queries] PSUM tiles.
  - exp split across ACT (true Exp activation -> fp8e4, bias -2.77) and
    DVE (Schraudolph: est_bits_u8 = round(1.4427*st + 23.571), saturating
    uint8 convert == fp8e4 bit pattern; bias/centering folded in).
  - PV with fp8 DoubleRow: two 128-key blocks per instruction (halves
    PE columns for PV).
  - normalize: DVE reciprocal_approx_fast of the denominator row,
    gpsimd partition_broadcast, one DVE mult -> ot fp8e4.
  - out proj: DoubleRow pairs heads 0,1; head 2 matmul carries a ones
    row whose rhs row is b_out (bias lands in PSUM); y copied out by
    ACT/DVE and DMA'd.
"""

import os
import sys
from contextlib import ExitStack

import numpy as np

for _p in ("/opt/trn_rl_repo", "/root/.axon_site/_ro/trn_rl_repo"):
    if os.path.isdir(_p) and _p not in sys.path:
        sys.path.append(_p)

import ml_dtypes
import concourse.bass as bass
import concourse.mybir as mybir
import concourse.tile as tile
from concourse import bacc
from concourse.bass import ts
from concourse.bass_utils import run_bass_kernel_spmd

F32 = mybir.dt.float32
F16 = mybir.dt.float16
BF16 = mybir.dt.bfloat16
FP8 = mybir.dt.float8e4
U8 = mybir.dt.uint8
U16 = mybir.dt.uint16

B, T, C = 2, 4096, 768
H, DK = 12, 64
N_CORES = 8
HPC = 3  # heads per core
GQ = 512  # q-block
NTQ = T // GQ  # 8
NTK = T // 128  # 32 key blocks
VP = 80  # padded vaug row length (65 used; 80 = 16B-aligned DR slot stride)

EXP_BIAS = -2.77  # est = exp(s + EXP_BIAS); denominator absorbs it
# fp16 Schraudolph: bits16 = round(st * A + B), minimax-centered (sigma=-44)
SCH_A = 0.125 * 1024 * 1.4426950408889634
SCH_B = 1024 * (15 + EXP_BIAS * 1.4426950408889634) - 44.0

AluOp = mybir.AluOpType
ActFn = mybir.ActivationFunctionType
DR = mybir.MatmulPerfMode.DoubleRow


def _build_program(taps=False):
    nc = bacc.Bacc("TRN2", target_bir_lowering=False, debug=False)

    xT = nc.dram_tensor("xT", [C, T], BF16, kind="ExternalInput").ap()
    # wqk: per d-chunk, col-pack cb0=[q0|q1] cb1=[k0|k1] cb2=[q2|k2]
    wqk = nc.dram_tensor("wqk", [C, 3, 128], BF16, kind="ExternalInput").ap()
    bqk = nc.dram_tensor("bqk", [3, 128], F32, kind="ExternalInput").ap()
    wv = nc.dram_tensor("wv", [C, 192], BF16, kind="ExternalInput").ap()
    bv = nc.dram_tensor("bv", [192], F32, kind="ExternalInput").ap()
    # wout row 64 of head-2 slice = b_out; rows 0:64 = W_out rows
    wout8 = nc.dram_tensor("wout8", [65, 3, C], F16, kind="ExternalInput").ap()
    y = nc.dram_tensor("y", [T, C], F32, kind="ExternalOutput").ap()
    dbg = {}
    if taps:
        for name, shape, dt in [
            ("dbg_qk01", [128, 2, 512], F16),
            ("dbg_q2", [64, 512], F16),
            ("dbg_k2", [64, 512], F16),
            ("dbg_vaug", [128, 3, 2, VP], F16),
            ("dbg_st", [128, 2, 512], F32),
            ("dbg_est", [128, 2, 512], F16),
            ("dbg_otps", [65, 512], F32),
            ("dbg_rc", [1, 512], F32),
            ("dbg_rb", [64, 512], F32),
            ("dbg_ot8", [65, 3, 512], F16),
            ("dbg_py", [128, 768], F32),
        ]:
            dbg[name] = nc.dram_tensor(name, shape, dt, kind="ExternalOutput").ap()

    xT3 = xT.rearrange("(o p) t -> p o t", p=128)  # [128, 6, 4096]
    wqk3 = wqk.rearrange("(o p) c k -> p o c k", p=128)  # [128, 6, 3, 128]
    wv3 = wv.rearrange("(o p) c -> p o c", p=128)  # [128, 6, 192]
    y3 = y.rearrange("(n p) e -> p n e", p=128)  # [128, 32, 768]

    with tile.TileContext(nc) as tc, ExitStack() as ctx:
        sb = ctx.enter_context(tc.tile_pool(name="persist", bufs=1))

        # --- weights / biases ---
        wqk_sb = sb.tile([128, 6, 3, 128], BF16)
        nc.sync.dma_start(wqk_sb[:], wqk3)
        wv_sb = sb.tile([128, 6, 192], BF16)
        nc.sync.dma_start(wv_sb[:], wv3)
        wout_sb = sb.tile([65, 3, C], F16)
        nc.sync.dma_start(wout_sb[:], wout8)
        bqk_sb = sb.tile([128, 3], F32)
        nc.sync.dma_start(bqk_sb[:], bqk.rearrange("c k -> k c"))
        bv_bc = sb.tile([128, 192], F32)
        nc.sync.dma_start(bv_bc[:], bv[None, :].to_broadcast((128, 192)))
        ebias = sb.tile([128, 1], F32)
        nc.vector.memset(ebias[:], EXP_BIAS)

        # --- persistent activations ---
        qk01 = sb.tile([128, 2, T], F16)  # slot0 q(h0|h1), slot1 k(h0|h1)
        q2 = sb.tile([64, T], F16)
        k2 = sb.tile([64, T], F16)
        vaug = sb.tile([128, HPC, NTK, VP], F16)
        nc.gpsimd.memset(vaug[:, :, :, 64:65], 1.0)

        # --- phase 1: qkv projections ---
        with (
            tc.tile_pool(name="psA", bufs=2, space="PSUM") as psA,
            tc.tile_pool(name="xin", bufs=2) as xin,
        ):
            for tb in range(NTQ):
                xt = xin.tile([128, 6, GQ], BF16, tag="x")
                nc.sync.dma_start(xt[:], xT3[:, :, ts(tb, GQ)])
                for cb in range(3):
                    ps = psA.tile([128, GQ], F32, tag="qk")
                    for d in range(6):
                        nc.tensor.matmul(
                            ps[:],
                            wqk_sb[:, d, cb, :],
                            xt[:, d, :],
                            start=(d == 0),
                            stop=(d == 5),
                        )
                    if cb < 2:
                        nc.scalar.activation(
                            qk01[:, cb, ts(tb, GQ)], ps[:], ActFn.Identity,
                            bias=bqk_sb[:, cb : cb + 1],
                        )
                    else:
                        nc.scalar.activation(
                            q2[:, ts(tb, GQ)], ps[0:64, :], ActFn.Identity,
                            bias=bqk_sb[0:64, 2:3],
                        )
                        nc.scalar.activation(
                            k2[:, ts(tb, GQ)], ps[64:128, :], ActFn.Identity,
                            bias=bqk_sb[64:128, 2:3],
                        )
                # v: token-major, 4 sub-blocks of 128 tokens
                for tsub in range(GQ // 128):
                    g = tb * 4 + tsub
                    psv = psA.tile([128, 192], F32, tag="v")
                    for d in range(6):
                        nc.tensor.matmul(
                            psv[:],
                            xt[:, d, ts(tsub, 128)],
                            wv_sb[:, d, :],
                            start=(d == 0),
                            stop=(d == 5),
                        )
                    nc.vector.tensor_tensor(
                        vaug[:, :, g, 0:64],
                        psv[:].rearrange("p (h e) -> p h e", e=64),
                        bv_bc[:].rearrange("p (h e) -> p h e", e=64),
                        AluOp.add,
                    )

        if taps:
            nc.sync.dma_start(dbg["dbg_qk01"], qk01[:, :, 0:512])
            nc.sync.dma_start(dbg["dbg_q2"], q2[:, 0:512])
            nc.sync.dma_start(dbg["dbg_k2"], k2[:, 0:512])
            nc.sync.dma_start(dbg["dbg_vaug"], vaug[:, :, 0:2, :])

        # --- phase 2: attention + output projection ---
        with (
            tc.tile_pool(name="psST", bufs=2, space="PSUM") as psST,
            tc.tile_pool(name="psOT", bufs=2, space="PSUM") as psOT,
            tc.tile_pool(name="psY", bufs=1, space="PSUM") as psY,
            tc.tile_pool(name="estp", bufs=3) as estp,
            tc.tile_pool(name="otp", bufs=2) as otp,
            tc.tile_pool(name="smallp", bufs=3) as smallp,
            tc.tile_pool(name="rbp", bufs=2) as rbp,
            tc.tile_pool(name="yp", bufs=2) as yp,
        ):
            eng_ctr = [0]

            EXP_MODE = int(os.environ.get("EXP_MODE", "0"))  # 0 alt, 1 ACT-only, 2 DVE-only

            def exp_op(est, st_ps):
                # alternate ACT true-exp and DVE Schraudolph
                if EXP_MODE == 1 or (EXP_MODE == 0 and eng_ctr[0] % 2 == 0):
                    nc.scalar.activation(
                        est[:], st_ps[:], ActFn.Exp, scale=0.125, bias=ebias[:]
                    )
                else:
                    nc.vector.tensor_scalar(
                        est[:].bitcast(U16), st_ps[:], SCH_A, SCH_B,
                        AluOp.mult, AluOp.add,
                    )
                eng_ctr[0] += 1

            for tq in range(NTQ):
                ot8 = otp.tile([65, HPC, GQ], F16, tag="ot")
                nc.gpsimd.memset(ot8[64:65, 2, :], 1.0)

                for h in range(HPC):
                    ot_ps = psOT.tile([65, GQ], F32, tag="ot")
                    for gp in range(NTK // 2):
                        ge, go = 2 * gp, 2 * gp + 1
                        st = psST.tile([128, 2, GQ], F32, tag="st")
                        if h < 2:
                            hb = 64 * h
                            nc.tensor.matmul(
                                st[:, 0, :],
                                qk01[hb : hb + 64, 1, ts(ge, 128)],
                                qk01[hb : hb + 64, 0, ts(tq, GQ)],
                                start=True, stop=True,
                            )
                            nc.tensor.matmul(
                                st[:, 1, :],
                                qk01[hb : hb + 64, 1, ts(go, 128)],
                                qk01[hb : hb + 64, 0, ts(tq, GQ)],
                                start=True, stop=True,
                            )
                        else:
                            nc.tensor.matmul(
                                st[:, 0, :],
                                k2[:, ts(ge, 128)],
                                q2[:, ts(tq, GQ)],
                                start=True, stop=True,
                            )
                            nc.tensor.matmul(
                                st[:, 1, :],
                                k2[:, ts(go, 128)],
                                q2[:, ts(tq, GQ)],
                                start=True, stop=True,
                            )
                        est = estp.tile([128, 2, GQ], F16, tag="est")
                        if taps and tq == 0 and h == 0 and gp == 0:
                            stc = estp.tile([128, 2, GQ], F32, tag="stc")
                            nc.scalar.copy(stc[:], st[:])
                            nc.sync.dma_start(dbg["dbg_st"], stc[:])
                        exp_op(est, st)
                        if taps and tq == 0 and h == 0 and gp == 0:
                            nc.sync.dma_start(dbg["dbg_est"], est[:])
                        nc.tensor.matmul(
                            ot_ps[:],
                            vaug[:, h, ge, 0:65],
                            est[:, 0, :],
                            start=(gp == 0),
                            stop=False,
                        )
                        nc.tensor.matmul(
                            ot_ps[:],
                            vaug[:, h, go, 0:65],
                            est[:, 1, :],
                            start=False,
                            stop=(gp == NTK // 2 - 1),
                        )
                    # normalize -> fp8 ot
                    dcp = smallp.tile([1, GQ], F32, tag="dcp")
                    nc.scalar.copy(dcp[:], ot_ps[64:65, :])
                    rc = smallp.tile([1, GQ], F32, tag="rc")
                    nc.vector.reciprocal_approx_fast(rc[:], dcp[:])
                    rb = rbp.tile([64, GQ], F32, tag="rb")
                    nc.gpsimd.partition_broadcast(rb[:], rc[:])
                    if taps and tq == 0 and h == 0:
                        otc = estp.tile([65, GQ], F32, tag="otc")
                        nc.scalar.copy(otc[:], ot_ps[:])
                        nc.sync.dma_start(dbg["dbg_otps"], otc[:])
                        nc.sync.dma_start(dbg["dbg_rc"], rc[:])
                        nc.sync.dma_start(dbg["dbg_rb"], rb[:])
                    nc.vector.tensor_tensor(
                        ot8[0:64, h, :], ot_ps[0:64, :], rb[:], AluOp.mult
                    )

                # output projection
                for tsub in range(GQ // 128):
                    py = psY.tile([128, 2, 384], F32, tag="y", padded_shape=[128, 2, 512])
                    for nh in range(2):
                        for hh in range(2):
                            nc.tensor.matmul(
                                py[:, nh, :],
                                ot8[0:64, hh, ts(tsub, 128)],
                                wout_sb[0:64, hh, ts(nh, 384)],
                                start=(hh == 0), stop=False,
                            )
                        nc.tensor.matmul(
                            py[:, nh, :],
                            ot8[0:65, 2, ts(tsub, 128)],
                            wout_sb[0:65, 2, ts(nh, 384)],
                            start=False, stop=True,
                        )
                    if taps and tq == 0 and tsub == 0:
                        nc.sync.dma_start(dbg["dbg_ot8"], ot8[:])
                        pyc = yp.tile([128, C], F32, tag="pyc")
                        nc.scalar.copy(pyc[:], py[:])
                        nc.sync.dma_start(dbg["dbg_py"], pyc[:])
                    y_sb = yp.tile([128, C], F32, tag="ysb")
                    if tsub % 2 == 0:
                        nc.scalar.copy(y_sb[:].rearrange("p (n e) -> p n e", n=2), py[:])
                    else:
                        nc.vector.tensor_copy(y_sb[:].rearrange("p (n e) -> p n e", n=2), py[:])
                    nc.sync.dma_start(y3[:, tq * 4 + tsub, :], y_sb[:])

    nc.compile()
    return nc


_PROGRAM = None


def _get_program():
    global _PROGRAM
    if _PROGRAM is None:
        _PROGRAM = _build_program()
    return _PROGRAM


def _make_in_maps(x, W_qkv, b_qkv, W_out, b_out):
    x = np.asarray(x, dtype=np.float32)
    W_qkv = np.asarray(W_qkv, dtype=np.float32)
    b_qkv = np.asarray(b_qkv, dtype=np.float32)
    W_out = np.asarray(W_out, dtype=np.float32)
    b_out = np.asarray(b_out, dtype=np.float32)

    xT_b = [np.ascontiguousarray(x[b].T).astype(ml_dtypes.bfloat16) for b in range(B)]
    in_maps = []
    for c in range(N_CORES):
        b, g = divmod(c, 4)
        h0 = HPC * g

        def qcol(h):
            return slice(h * DK, (h + 1) * DK)

        def kcol(h):
            return slice(C + h * DK, C + (h + 1) * DK)

        # wqk [C, 3, 128]: cb0=[q0|q1] cb1=[k0|k1] cb2=[q2|k2]
        wqk_c = np.stack(
            [
                np.concatenate([W_qkv[:, qcol(h0)], W_qkv[:, qcol(h0 + 1)]], axis=1),
                np.concatenate([W_qkv[:, kcol(h0)], W_qkv[:, kcol(h0 + 1)]], axis=1),
                np.concatenate([W_qkv[:, qcol(h0 + 2)], W_qkv[:, kcol(h0 + 2)]], axis=1),
            ],
            axis=1,
        )
        bqk_c = np.stack(
            [
                np.concatenate([b_qkv[qcol(h0)], b_qkv[qcol(h0 + 1)]]),
                np.concatenate([b_qkv[kcol(h0)], b_qkv[kcol(h0 + 1)]]),
                np.concatenate([b_qkv[qcol(h0 + 2)], b_qkv[kcol(h0 + 2)]]),
            ],
            axis=0,
        )
        vs = slice(2 * C + h0 * DK, 2 * C + (h0 + HPC) * DK)

        wout_c = np.zeros((65, 3, C), dtype=np.float32)
        for h in range(HPC):
            wout_c[0:64, h, :] = W_out[(h0 + h) * DK : (h0 + h + 1) * DK, :]
        if g == 0:
            wout_c[64, 2, :] = b_out  # bias rides head-2's ones row
        wout_c8 = wout_c.astype(np.float16)

        in_maps.append(
            {
                "xT": xT_b[b],
                "wqk": np.ascontiguousarray(wqk_c).astype(ml_dtypes.bfloat16),
                "bqk": np.ascontiguousarray(bqk_c),
                "wv": np.ascontiguousarray(W_qkv[:, vs]).astype(ml_dtypes.bfloat16),
                "bv": np.ascontiguousarray(b_qkv[vs]),
                "wout8": wout_c8,
            }
        )
    return in_maps


def _assemble(results):
    out = np.zeros((B, T, C), dtype=np.float32)
    for c in range(N_CORES):
        out[c // 4] += results[c]["y"]
    return out


def kernel_run(inputs, trace=False):
    """Returns (full_output [B,T,C] fp32, exec_time_ns or None)."""
    nc = _get_program()
    in_maps = _make_in_maps(**inputs)
    res = run_bass_kernel_spmd(
        nc, in_maps, core_ids=list(range(N_CORES)), trace=trace
    )
    return _assemble(res.results), res.exec_time_ns


def kernel(**inputs):
    out, _ = kernel_run(inputs)
    return out
